# revision 7
# baseline (speedup 1.0000x reference)
"""Masked attention kernel for Trainium2, 8 NeuronCores.

Problem: q,k,v [32,1024,64] f32, mask [32,1024,1024] bool (True -> -inf),
out = softmax(q@k^T * D^-0.5 masked) @ v.

Sharding: batch*head dim (32) split across 8 cores, 4 heads/core.

Per-core device algorithm (T-layout):
  scoresT[t,s] = sum_d k[t,d] q[s,d]  computed via PE with
      lhsT = kT [64,128] chunk, rhs = qT [64,1024]  (host provides qT/kT)
  mask added in PSUM via PE:  scoresT += (-240*I128) @ mT  (mask fp8, host-transposed)
      exp(0.125*(s-240)) <= e^-24  -> masked lanes vanish.
  pT = exp(0.125 * scoresT)  on ACT (no row-max needed: |s|*0.125 <= ~6)
  outT_aug[d,s] = sum_t v_aug[t,d] pT[t,s]  with v_aug = [v | ones] so row 64
      carries the softmax denominators.
  out = outT / sums  (reciprocal computed in a transposed [128,8] layout to
      dodge the 8-cycle/elem reciprocal cost, broadcast via tiny DMA bounce)
Host does all pure-layout work: q/k/mask transposes, fp8 cast, final
outT -> out transpose.
"""

import os
import sys

import numpy as np

for _p in ("/opt/trn_rl_repo", "/opt/pypackages"):
    if _p not in sys.path and os.path.isdir(_p):
        sys.path.append(_p)

import ml_dtypes  # noqa: E402

import concourse.bass as bass  # noqa: E402
import concourse.tile as tile  # noqa: E402
from concourse import mybir  # noqa: E402
from concourse.bass_utils import run_bass_kernel_spmd  # noqa: E402

BH, S, D = 32, 1024, 64
NCORES = 8
HPC = BH // NCORES  # heads per core
NT = S // 128  # 8 tiles of 128 along s/t
FP8 = ml_dtypes.float8_e4m3fn
F32 = mybir.dt.float32
DT8 = mybir.dt.float8e4
MASK_NEG = -240.0  # exp(0.125*(-240)) ~ 9e-14; representable in fp8e4m3


def _build_program():
    nc = bass.Bass(
        "TRN2",
        target_bir_lowering=False,
        debug=False,
        num_devices=NCORES,
    )
    qkt = nc.dram_tensor("qkt", [HPC, 64, 2 * S], F32, kind="ExternalInput").ap()
    vaug = nc.dram_tensor("vaug", [HPC, S, 65], F32, kind="ExternalInput").ap()
    mt8 = nc.dram_tensor("mt8", [HPC, S, S], DT8, kind="ExternalInput").ap()
    negi = nc.dram_tensor("negi", [128, 128], DT8, kind="ExternalInput").ap()
    ident = nc.dram_tensor("ident", [128, 128], F32, kind="ExternalInput").ap()
    outt = nc.dram_tensor("outt", [HPC, 64, S], F32, kind="ExternalOutput").ap()

    with tile.TileContext(nc) as tc:
        with (
            tc.tile_pool(name="const", bufs=1) as const_pool,
            tc.tile_pool(name="qk", bufs=2) as qk_pool,
            tc.tile_pool(name="v", bufs=2) as v_pool,
            tc.tile_pool(name="m", bufs=2) as m_pool,
            tc.tile_pool(name="p", bufs=2) as p_pool,
            tc.tile_pool(name="ot", bufs=2) as ot_pool,
            tc.tile_pool(name="fin", bufs=2) as fin_pool,
            tc.tile_pool(name="spsum", bufs=2, space="PSUM") as s_pool,
            tc.tile_pool(name="opsum", bufs=1, space="PSUM") as o_pool,
            tc.tile_pool(name="tpsum", bufs=1, space="PSUM") as t_pool,
            tc.tile_pool(name="rtpsum", bufs=1, space="PSUM") as rt_pool,
            tc.tile_pool(name="dram", bufs=2, space="DRAM") as dram_pool,
        ):
            negi_sb = const_pool.tile([128, 128], DT8)
            nc.sync.dma_start(negi_sb[:], negi[:])
            ident_sb = const_pool.tile([128, 128], F32)
            nc.sync.dma_start(ident_sb[:], ident[:])
            ones65_sb = const_pool.tile([65, 1], F32)
            nc.gpsimd.memset(ones65_sb[:], 1.0)

            for h in range(HPC):
                qk_sb = qk_pool.tile([64, 2 * S], F32)
                nc.sync.dma_start(qk_sb[:], qkt[h])
                v_sb = v_pool.tile([128, NT * 65], F32)
                nc.sync.dma_start(
                    v_sb[:].rearrange("p (T d) -> p T d", T=NT),
                    vaug[h].rearrange("(T p) d -> p T d", p=128),
                )
                m_sb = m_pool.tile([128, NT * S], DT8)
                nc.sync.dma_start(
                    m_sb[:].rearrange("p (T s) -> p T s", T=NT),
                    mt8[h].rearrange("(T p) s -> p T s", p=128),
                )

                p_sb = p_pool.tile([128, NT * S], F32)
                for t in range(NT):
                    s_ps = s_pool.tile([128, S], F32)
                    kslc = slice(S + t * 128, S + (t + 1) * 128)
                    # QK^T (transposed scores): both 512-wide halves with one
                    # weight load, then the mask accumulation with one more.
                    for n in range(2):
                        sl = slice(n * 512, (n + 1) * 512)
                        nc.tensor.matmul(
                            out=s_ps[:, sl],
                            lhsT=qk_sb[:, kslc],
                            rhs=qk_sb[:, sl],
                            start=True,
                            stop=False,
                        )
                    for n in range(2):
                        sl = slice(n * 512, (n + 1) * 512)
                        nc.tensor.matmul(
                            out=s_ps[:, sl],
                            lhsT=negi_sb[:],
                            rhs=m_sb[:, t * S + n * 512 : t * S + (n + 1) * 512],
                            start=False,
                            stop=True,
                        )
                    nc.scalar.activation(
                        out=p_sb[:, t * S : (t + 1) * S],
                        in_=s_ps[:],
                        func=mybir.ActivationFunctionType.Exp,
                        scale=0.125,
                    )

                # out^T augmented with the denominator row (65 = 64 dims + sum)
                o_ps = o_pool.tile([65, S], F32)
                for t in range(NT):
                    for n in range(2):
                        sl = slice(n * 512, (n + 1) * 512)
                        nc.tensor.matmul(
                            out=o_ps[:, sl],
                            lhsT=v_sb[:, t * 65 : (t + 1) * 65],
                            rhs=p_sb[:, t * S + n * 512 : t * S + (n + 1) * 512],
                            start=(t == 0),
                            stop=(t == NT - 1),
                        )
                ot_sb = ot_pool.tile([65, S], F32)
                nc.vector.tensor_copy(ot_sb[:], o_ps[:])

                # transpose sums [1,1024] -> [128,8] via K=1 matmuls so the
                # expensive (8 cyc/elem) reciprocal touches only 8 elems/lane
                t_ps = t_pool.tile([128, NT], F32)
                for j in range(NT):
                    nc.tensor.matmul(
                        out=t_ps[:, j : j + 1],
                        lhsT=ot_sb[64:65, j * 128 : (j + 1) * 128],
                        rhs=ones65_sb[64:65, :],
                        start=True,
                        stop=True,
                    )
                r_sb = fin_pool.tile([128, NT], F32, tag="rsb")
                nc.vector.reciprocal(r_sb[:], t_ps[:])
                # transpose back to [8,128] and broadcast to 64 partitions
                rt_ps = rt_pool.tile([NT, 128], F32)
                nc.tensor.matmul(
                    out=rt_ps[:],
                    lhsT=r_sb[:],
                    rhs=ident_sb[:],
                    start=True,
                    stop=True,
                )
                rt_sb = fin_pool.tile([NT, 128], F32, tag="rtsb")
                nc.vector.tensor_copy(rt_sb[:], rt_ps[:])
                r_dram = dram_pool.tile([NT, 128], F32)
                nc.sync.dma_start(r_dram[:], rt_sb[:])
                rrep_sb = fin_pool.tile([64, S], F32, tag="rrep")
                nc.sync.dma_start(
                    rrep_sb[:],
                    r_dram[:].rearrange("p f -> () (p f)").to_broadcast((64, S)),
                )
                ots_sb = fin_pool.tile([64, S], F32, tag="ots")
                nc.vector.tensor_mul(out=ots_sb[:], in0=ot_sb[0:64, :], in1=rrep_sb[:])
                nc.sync.dma_start(outt[h], ots_sb[:])

    _split_multi_waits(nc)
    return nc


def _split_multi_waits(nc):
    """Walrus's S3_LW codegen can't take >1 sync-wait condition on a Matmult;
    hoist extras into standalone EventSemaphore instructions (same semantics:
    the engine queue stalls on them in program order, like raw-bass wait_ge)."""
    for bb in nc.bb_map.values():
        insts = bb.bb.instructions
        new_list = []
        for inst in insts:
            si = getattr(inst, "sync_info", None)
            if (
                si is not None
                and si.on_wait
                and len(si.on_wait) > 1
            ):
                extra = si.on_wait[:-1]
                keep = si.on_wait[-1:]
                for cond in extra:
                    new_list.append(
                        mybir.InstEventSemaphore(
                            name=nc.get_next_instruction_name(),
                            ins=[],
                            outs=[],
                            engine=inst.engine,
                            sync_info=mybir.SyncInfo(on_wait=[cond], on_update=[]),
                        )
                    )
                si.on_wait = keep
            new_list.append(inst)
        insts[:] = new_list


_NC_CACHE = None


def _get_nc():
    global _NC_CACHE
    if _NC_CACHE is None:
        _NC_CACHE = _build_program()
    return _NC_CACHE


def _make_in_maps(q, k, v, mask):
    q = np.ascontiguousarray(np.asarray(q, dtype=np.float32))
    k = np.ascontiguousarray(np.asarray(k, dtype=np.float32))
    v = np.ascontiguousarray(np.asarray(v, dtype=np.float32))
    mask = np.asarray(mask)
    negi_np = (np.eye(128, dtype=np.float32) * MASK_NEG).astype(FP8)
    ident_np = np.eye(128, dtype=np.float32)
    ones_col = np.ones((HPC, S, 1), dtype=np.float32)
    in_maps = []
    for c in range(NCORES):
        sl = slice(c * HPC, (c + 1) * HPC)
        qT = q[sl].transpose(0, 2, 1)  # [HPC, 64, S]
        kT = k[sl].transpose(0, 2, 1)
        qkt_np = np.ascontiguousarray(np.concatenate([qT, kT], axis=2))
        vaug_np = np.ascontiguousarray(np.concatenate([v[sl], ones_col], axis=2))
        mt8_np = np.ascontiguousarray(
            mask[sl].transpose(0, 2, 1).astype(np.float32)
        ).astype(FP8)
        in_maps.append(
            {
                "qkt": qkt_np,
                "vaug": vaug_np,
                "mt8": mt8_np,
                "negi": negi_np,
                "ident": ident_np,
            }
        )
    return in_maps


def _gather(results):
    outs = []
    for c in range(NCORES):
        outT = np.asarray(results[c]["outt"], dtype=np.float32)  # [HPC, 64, S]
        outs.append(outT.transpose(0, 2, 1))  # [HPC, S, 64]
    return np.ascontiguousarray(np.concatenate(outs, axis=0))


def _install_profile_shim():
    """The agent image's antenv lacks axon_hooks; recreate it from the boot
    module's ctypes implementation so trace=True can capture NTFF profiles."""
    import types

    if "antenv.axon_hooks" in sys.modules:
        return
    try:
        from trn_agent_boot.trn_boot import _ntff_profile_via_ctypes

        hook = _ntff_profile_via_ctypes("/opt/axon/libaxon_pjrt.so")
        mod = types.ModuleType("antenv.axon_hooks")
        mod.get_axon_ntff_profile_hook = lambda: hook
        mod.set_axon_ntff_profile_hook = lambda h: None
        sys.modules["antenv.axon_hooks"] = mod
        # don't try to copy artifacts to a remote bucket from the sandbox
        import concourse.bass_utils as _bu

        _bu.upload_artifacts = lambda tmpdir: tmpdir
    except Exception as e:  # profiling is best-effort
        print(f"profile shim unavailable: {e}", file=sys.stderr)


def run(q, k, v, mask, trace=False, **kw):
    nc = _get_nc()
    if trace:
        _install_profile_shim()
    in_maps = _make_in_maps(q, k, v, mask)
    res = run_bass_kernel_spmd(nc, in_maps, list(range(NCORES)), trace=trace, **kw)
    return _gather(res.results), res


def kernel(q, k, v, mask):
    out, _ = run(q, k, v, mask)
    return out


# revision 9
# speedup vs baseline: 1.9663x; 1.9663x over previous
"""Masked attention kernel for Trainium2, 8 NeuronCores.

Problem: q,k,v [32,1024,64] f32, mask [32,1024,1024] bool (True -> -inf),
out = softmax(q@k^T * D^-0.5 masked) @ v.

Sharding: batch*head dim (32) split across 8 cores, 4 heads/core.

Per-core device algorithm (T-layout):
  scoresT[t,s] = sum_d k[t,d] q[s,d]  computed via PE with
      lhsT = kT [64,128] chunk, rhs = qT [64,1024]  (host provides qT/kT)
  mask added in PSUM via PE:  scoresT += (-240*I128) @ mT  (mask fp8, host-transposed)
      exp(0.125*(s-240)) <= e^-24  -> masked lanes vanish.
  pT = exp(0.125 * scoresT)  on ACT (no row-max needed: |s|*0.125 <= ~6)
  outT_aug[d,s] = sum_t v_aug[t,d] pT[t,s]  with v_aug = [v | ones] so row 64
      carries the softmax denominators.
  out = outT / sums  (reciprocal computed in a transposed [128,8] layout to
      dodge the 8-cycle/elem reciprocal cost, broadcast via tiny DMA bounce)
Host does all pure-layout work: q/k/mask transposes, fp8 cast, final
outT -> out transpose.
"""

import os
import sys

import numpy as np

for _p in ("/opt/trn_rl_repo", "/opt/pypackages"):
    if _p not in sys.path and os.path.isdir(_p):
        sys.path.append(_p)

import ml_dtypes  # noqa: E402

import concourse.bass as bass  # noqa: E402
import concourse.tile as tile  # noqa: E402
from concourse import mybir  # noqa: E402
from concourse.bass_utils import run_bass_kernel_spmd  # noqa: E402

BH, S, D = 32, 1024, 64
NCORES = 8
HPC = BH // NCORES  # heads per core
NT = S // 128  # 8 tiles of 128 along s/t
FP8 = ml_dtypes.float8_e4m3fn
F32 = mybir.dt.float32
BF16 = mybir.dt.bfloat16
DT8 = mybir.dt.float8e4
MASK_NEG = -240.0  # exp(0.125*(-240)) ~ 9e-14; representable in fp8e4m3


def _build_program():
    nc = bass.Bass(
        "TRN2",
        target_bir_lowering=False,
        debug=False,
        num_devices=NCORES,
    )
    qkt = nc.dram_tensor("qkt", [HPC, 64, 2 * S], BF16, kind="ExternalInput").ap()
    vaug = nc.dram_tensor("vaug", [HPC, S, 65], BF16, kind="ExternalInput").ap()
    mt8 = nc.dram_tensor("mt8", [HPC, S, S], DT8, kind="ExternalInput").ap()
    negi = nc.dram_tensor("negi", [128, 128], DT8, kind="ExternalInput").ap()
    outt = nc.dram_tensor("outt", [HPC, 64, S], F32, kind="ExternalOutput").ap()

    with tile.TileContext(nc) as tc:
        with (
            tc.tile_pool(name="const", bufs=1) as const_pool,
            tc.tile_pool(name="qk", bufs=2) as qk_pool,
            tc.tile_pool(name="v", bufs=2) as v_pool,
            tc.tile_pool(name="m", bufs=2) as m_pool,
            tc.tile_pool(name="p", bufs=2) as p_pool,
            tc.tile_pool(name="ot", bufs=2) as ot_pool,
            tc.tile_pool(name="fin", bufs=2) as fin_pool,
            tc.tile_pool(name="spsum", bufs=3, space="PSUM") as s_pool,
            tc.tile_pool(name="opsum", bufs=1, space="PSUM") as o_pool,
            tc.tile_pool(name="dram", bufs=2, space="DRAM") as dram_pool,
        ):
            negi_sb = const_pool.tile([128, 128], DT8)
            nc.sync.dma_start(negi_sb[:], negi[:])

            for h in range(HPC):
                qk_sb = qk_pool.tile([64, 2 * S], BF16)
                nc.sync.dma_start(qk_sb[:], qkt[h])
                v_sb = v_pool.tile([128, NT * 65], BF16)
                nc.sync.dma_start(
                    v_sb[:].rearrange("p (T d) -> p T d", T=NT),
                    vaug[h].rearrange("(T p) d -> p T d", p=128),
                )
                m_sb = m_pool.tile([128, NT * S], DT8)
                nc.sync.dma_start(
                    m_sb[:].rearrange("p (T s) -> p T s", T=NT),
                    mt8[h].rearrange("(T p) s -> p T s", p=128),
                )

                p_sb = p_pool.tile([128, NT * S], BF16)
                for t in range(NT):
                    s_ps = s_pool.tile([128, S], F32)
                    kslc = slice(S + t * 128, S + (t + 1) * 128)
                    # QK^T (transposed scores): both 512-wide halves with one
                    # weight load, then the mask accumulation with one more.
                    for n in range(2):
                        sl = slice(n * 512, (n + 1) * 512)
                        nc.tensor.matmul(
                            out=s_ps[:, sl],
                            lhsT=qk_sb[:, kslc],
                            rhs=qk_sb[:, sl],
                            start=True,
                            stop=False,
                        )
                    for n in range(2):
                        sl = slice(n * 512, (n + 1) * 512)
                        nc.tensor.matmul(
                            out=s_ps[:, sl],
                            lhsT=negi_sb[:],
                            rhs=m_sb[:, t * S + n * 512 : t * S + (n + 1) * 512],
                            start=False,
                            stop=True,
                        )
                    nc.scalar.activation(
                        out=p_sb[:, t * S : (t + 1) * S],
                        in_=s_ps[:],
                        func=mybir.ActivationFunctionType.Exp,
                        scale=0.125,
                    )

                # out^T augmented with the denominator row (65 = 64 dims + sum)
                o_ps = o_pool.tile([65, S], F32)
                for t in range(NT):
                    for n in range(2):
                        sl = slice(n * 512, (n + 1) * 512)
                        nc.tensor.matmul(
                            out=o_ps[:, sl],
                            lhsT=v_sb[:, t * 65 : (t + 1) * 65],
                            rhs=p_sb[:, t * S + n * 512 : t * S + (n + 1) * 512],
                            start=(t == 0),
                            stop=(t == NT - 1),
                        )
                ot_sb = ot_pool.tile([65, S], F32)
                nc.vector.tensor_copy(ot_sb[:], o_ps[:])

                # sums [1,1024] -> DRAM -> [128,8] (transposed via DMA strides)
                # so the 8-cyc/elem reciprocal runs 128-wide, then bounce the
                # reciprocals back to a flat row and broadcast-replicate.
                sums_dram = dram_pool.tile([1, S], F32, tag="sums")
                nc.sync.dma_start(sums_dram[:], ot_sb[64:65, :])
                rt_sb = fin_pool.tile([128, NT], F32, tag="rtsb")
                nc.sync.dma_start(
                    rt_sb[:], sums_dram[0, :].rearrange("(j p) -> p j", p=128)
                )
                r_sb = fin_pool.tile([128, NT], F32, tag="rsb")
                nc.vector.reciprocal(r_sb[:], rt_sb[:])
                r_dram = dram_pool.tile([1, S], F32, tag="rdram")
                nc.sync.dma_start(
                    r_dram[0, :].rearrange("(j p) -> p j", p=128), r_sb[:]
                )
                rrep_sb = fin_pool.tile([64, S], F32, tag="rrep")
                nc.sync.dma_start(
                    rrep_sb[:], r_dram[:].to_broadcast((64, S))
                )
                ots_sb = fin_pool.tile([64, S], F32, tag="ots")
                nc.vector.tensor_mul(out=ots_sb[:], in0=ot_sb[0:64, :], in1=rrep_sb[:])
                nc.sync.dma_start(outt[h], ots_sb[:])

    _split_multi_waits(nc)
    return nc


def _split_multi_waits(nc):
    """Walrus's S3_LW codegen can't take >1 sync-wait condition on a Matmult;
    hoist extras into standalone EventSemaphore instructions (same semantics:
    the engine queue stalls on them in program order, like raw-bass wait_ge)."""
    for bb in nc.bb_map.values():
        insts = bb.bb.instructions
        new_list = []
        for inst in insts:
            si = getattr(inst, "sync_info", None)
            if (
                si is not None
                and si.on_wait
                and len(si.on_wait) > 1
            ):
                extra = si.on_wait[:-1]
                keep = si.on_wait[-1:]
                for cond in extra:
                    new_list.append(
                        mybir.InstEventSemaphore(
                            name=nc.get_next_instruction_name(),
                            ins=[],
                            outs=[],
                            engine=inst.engine,
                            sync_info=mybir.SyncInfo(on_wait=[cond], on_update=[]),
                        )
                    )
                si.on_wait = keep
            new_list.append(inst)
        insts[:] = new_list


_NC_CACHE = None


def _get_nc():
    global _NC_CACHE
    if _NC_CACHE is None:
        _NC_CACHE = _build_program()
    return _NC_CACHE


def _make_in_maps(q, k, v, mask):
    q = np.ascontiguousarray(np.asarray(q, dtype=np.float32))
    k = np.ascontiguousarray(np.asarray(k, dtype=np.float32))
    v = np.ascontiguousarray(np.asarray(v, dtype=np.float32))
    mask = np.asarray(mask)
    negi_np = (np.eye(128, dtype=np.float32) * MASK_NEG).astype(FP8)
    ones_col = np.ones((HPC, S, 1), dtype=np.float32)
    in_maps = []
    for c in range(NCORES):
        sl = slice(c * HPC, (c + 1) * HPC)
        qT = q[sl].transpose(0, 2, 1)  # [HPC, 64, S]
        kT = k[sl].transpose(0, 2, 1)
        qkt_np = np.ascontiguousarray(np.concatenate([qT, kT], axis=2)).astype(ml_dtypes.bfloat16)
        vaug_np = np.ascontiguousarray(np.concatenate([v[sl], ones_col], axis=2)).astype(ml_dtypes.bfloat16)
        mt8_np = np.ascontiguousarray(
            mask[sl].transpose(0, 2, 1).astype(np.float32)
        ).astype(FP8)
        in_maps.append(
            {
                "qkt": qkt_np,
                "vaug": vaug_np,
                "mt8": mt8_np,
                "negi": negi_np,
            }
        )
    return in_maps


def _gather(results):
    outs = []
    for c in range(NCORES):
        outT = np.asarray(results[c]["outt"], dtype=np.float32)  # [HPC, 64, S]
        outs.append(outT.transpose(0, 2, 1))  # [HPC, S, 64]
    return np.ascontiguousarray(np.concatenate(outs, axis=0))


def _install_profile_shim():
    """The agent image's antenv lacks axon_hooks; recreate it from the boot
    module's ctypes implementation so trace=True can capture NTFF profiles."""
    import types

    if "antenv.axon_hooks" in sys.modules:
        return
    try:
        from trn_agent_boot.trn_boot import _ntff_profile_via_ctypes

        hook = _ntff_profile_via_ctypes("/opt/axon/libaxon_pjrt.so")
        mod = types.ModuleType("antenv.axon_hooks")
        mod.get_axon_ntff_profile_hook = lambda: hook
        mod.set_axon_ntff_profile_hook = lambda h: None
        sys.modules["antenv.axon_hooks"] = mod
        # don't try to copy artifacts to a remote bucket from the sandbox
        import concourse.bass_utils as _bu

        _bu.upload_artifacts = lambda tmpdir: tmpdir
    except Exception as e:  # profiling is best-effort
        print(f"profile shim unavailable: {e}", file=sys.stderr)


def run(q, k, v, mask, trace=False, **kw):
    nc = _get_nc()
    if trace:
        _install_profile_shim()
    in_maps = _make_in_maps(q, k, v, mask)
    res = run_bass_kernel_spmd(nc, in_maps, list(range(NCORES)), trace=trace, **kw)
    return _gather(res.results), res


def kernel(q, k, v, mask):
    out, _ = run(q, k, v, mask)
    return out


# revision 10
# speedup vs baseline: 2.0547x; 1.0450x over previous
"""Masked attention kernel for Trainium2, 8 NeuronCores.

Problem: q,k,v [32,1024,64] f32, mask [32,1024,1024] bool (True -> -inf),
out = softmax(q@k^T * D^-0.5 masked) @ v.

Sharding: batch*head dim (32) split across 8 cores, 4 heads/core.

Per-core device algorithm (T-layout):
  scoresT[t,s] = sum_d k[t,d] q[s,d]  computed via PE with
      lhsT = kT [64,128] chunk, rhs = qT [64,1024]  (host provides qT/kT)
  mask added in PSUM via PE:  scoresT += (-240*I128) @ mT  (mask fp8, host-transposed)
      exp(0.125*(s-240)) <= e^-24  -> masked lanes vanish.
  pT = exp(0.125 * scoresT)  on ACT (no row-max needed: |s|*0.125 <= ~6)
  outT_aug[d,s] = sum_t v_aug[t,d] pT[t,s]  with v_aug = [v | ones] so row 64
      carries the softmax denominators.
  out = outT / sums  (reciprocal computed in a transposed [128,8] layout to
      dodge the 8-cycle/elem reciprocal cost, broadcast via tiny DMA bounce)
Host does all pure-layout work: q/k/mask transposes, fp8 cast, final
outT -> out transpose.
"""

import os
import sys

import numpy as np

for _p in ("/opt/trn_rl_repo", "/opt/pypackages"):
    if _p not in sys.path and os.path.isdir(_p):
        sys.path.append(_p)

import ml_dtypes  # noqa: E402

import concourse.bass as bass  # noqa: E402
import concourse.tile as tile  # noqa: E402
from concourse import mybir  # noqa: E402
from concourse.bass_utils import run_bass_kernel_spmd  # noqa: E402

BH, S, D = 32, 1024, 64
NCORES = 8
HPC = BH // NCORES  # heads per core
NT = S // 128  # 8 tiles of 128 along s/t
FP8 = ml_dtypes.float8_e4m3fn
F32 = mybir.dt.float32
BF16 = mybir.dt.bfloat16
DT8 = mybir.dt.float8e4
MASK_NEG = -240.0  # exp(0.125*(-240)) ~ 9e-14; representable in fp8e4m3


def _build_program():
    nc = bass.Bass(
        "TRN2",
        target_bir_lowering=False,
        debug=False,
        num_devices=NCORES,
    )
    qkt = nc.dram_tensor("qkt", [HPC, 64, 2 * S], BF16, kind="ExternalInput").ap()
    vaug = nc.dram_tensor("vaug", [HPC, S, 65], BF16, kind="ExternalInput").ap()
    mt8 = nc.dram_tensor("mt8", [HPC, S, S], DT8, kind="ExternalInput").ap()
    negi = nc.dram_tensor("negi", [128, 128], DT8, kind="ExternalInput").ap()
    outt = nc.dram_tensor("outt", [HPC, 64, S], F32, kind="ExternalOutput").ap()

    with tile.TileContext(nc) as tc:
        with (
            tc.tile_pool(name="const", bufs=1) as const_pool,
            tc.tile_pool(name="qk", bufs=HPC) as qk_pool,
            tc.tile_pool(name="v", bufs=HPC) as v_pool,
            tc.tile_pool(name="m", bufs=HPC) as m_pool,
            tc.tile_pool(name="p", bufs=2) as p_pool,
            tc.tile_pool(name="ot", bufs=2) as ot_pool,
            tc.tile_pool(name="fin", bufs=2) as fin_pool,
            tc.tile_pool(name="spsum", bufs=3, space="PSUM") as s_pool,
            tc.tile_pool(name="opsum", bufs=1, space="PSUM") as o_pool,
            tc.tile_pool(name="dram", bufs=2, space="DRAM") as dram_pool,
        ):
            negi_sb = const_pool.tile([128, 128], DT8)
            nc.sync.dma_start(negi_sb[:], negi[:])

            qk_tiles, v_tiles, m_tiles = [], [], []
            for h in range(HPC):
                qk_sb = qk_pool.tile([64, 2 * S], BF16)
                nc.sync.dma_start(qk_sb[:], qkt[h])
                v_sb = v_pool.tile([128, NT * 65], BF16)
                nc.sync.dma_start(
                    v_sb[:].rearrange("p (T d) -> p T d", T=NT),
                    vaug[h].rearrange("(T p) d -> p T d", p=128),
                )
                m_sb = m_pool.tile([128, NT * S], DT8)
                nc.sync.dma_start(
                    m_sb[:].rearrange("p (T s) -> p T s", T=NT),
                    mt8[h].rearrange("(T p) s -> p T s", p=128),
                )
                qk_tiles.append(qk_sb)
                v_tiles.append(v_sb)
                m_tiles.append(m_sb)

            for h in range(HPC):
                qk_sb, v_sb, m_sb = qk_tiles[h], v_tiles[h], m_tiles[h]
                p_sb = p_pool.tile([128, NT * S], BF16)
                for t in range(NT):
                    s_ps = s_pool.tile([128, S], F32)
                    kslc = slice(S + t * 128, S + (t + 1) * 128)
                    # QK^T (transposed scores): both 512-wide halves with one
                    # weight load, then the mask accumulation with one more.
                    for n in range(2):
                        sl = slice(n * 512, (n + 1) * 512)
                        nc.tensor.matmul(
                            out=s_ps[:, sl],
                            lhsT=qk_sb[:, kslc],
                            rhs=qk_sb[:, sl],
                            start=True,
                            stop=False,
                        )
                    for n in range(2):
                        sl = slice(n * 512, (n + 1) * 512)
                        nc.tensor.matmul(
                            out=s_ps[:, sl],
                            lhsT=negi_sb[:],
                            rhs=m_sb[:, t * S + n * 512 : t * S + (n + 1) * 512],
                            start=False,
                            stop=True,
                        )
                    nc.scalar.activation(
                        out=p_sb[:, t * S : (t + 1) * S],
                        in_=s_ps[:],
                        func=mybir.ActivationFunctionType.Exp,
                        scale=0.125,
                    )

                # out^T augmented with the denominator row (65 = 64 dims + sum)
                o_ps = o_pool.tile([65, S], F32)
                for t in range(NT):
                    for n in range(2):
                        sl = slice(n * 512, (n + 1) * 512)
                        nc.tensor.matmul(
                            out=o_ps[:, sl],
                            lhsT=v_sb[:, t * 65 : (t + 1) * 65],
                            rhs=p_sb[:, t * S + n * 512 : t * S + (n + 1) * 512],
                            start=(t == 0),
                            stop=(t == NT - 1),
                        )
                ot_sb = ot_pool.tile([65, S], F32)
                nc.vector.tensor_copy(ot_sb[:], o_ps[:])

                # sums [1,1024] -> DRAM -> [128,8] (transposed via DMA strides)
                # so the 8-cyc/elem reciprocal runs 128-wide, then bounce the
                # reciprocals back to a flat row and broadcast-replicate.
                sums_dram = dram_pool.tile([1, S], F32, tag="sums")
                nc.gpsimd.dma_start(sums_dram[:], ot_sb[64:65, :])
                rt_sb = fin_pool.tile([128, NT], F32, tag="rtsb")
                nc.gpsimd.dma_start(
                    rt_sb[:], sums_dram[0, :].rearrange("(j p) -> p j", p=128)
                )
                r_sb = fin_pool.tile([128, NT], F32, tag="rsb")
                nc.vector.reciprocal(r_sb[:], rt_sb[:])
                r_dram = dram_pool.tile([1, S], F32, tag="rdram")
                nc.gpsimd.dma_start(
                    r_dram[0, :].rearrange("(j p) -> p j", p=128), r_sb[:]
                )
                rrep_sb = fin_pool.tile([64, S], F32, tag="rrep")
                nc.gpsimd.dma_start(
                    rrep_sb[:], r_dram[:].to_broadcast((64, S))
                )
                ots_sb = fin_pool.tile([64, S], F32, tag="ots")
                nc.vector.tensor_mul(out=ots_sb[:], in0=ot_sb[0:64, :], in1=rrep_sb[:])
                nc.sync.dma_start(outt[h], ots_sb[:])

    _split_multi_waits(nc)
    return nc


def _split_multi_waits(nc):
    """Walrus's S3_LW codegen can't take >1 sync-wait condition on a Matmult;
    hoist extras into standalone EventSemaphore instructions (same semantics:
    the engine queue stalls on them in program order, like raw-bass wait_ge)."""
    for bb in nc.bb_map.values():
        insts = bb.bb.instructions
        new_list = []
        for inst in insts:
            si = getattr(inst, "sync_info", None)
            if (
                si is not None
                and si.on_wait
                and len(si.on_wait) > 1
            ):
                extra = si.on_wait[:-1]
                keep = si.on_wait[-1:]
                for cond in extra:
                    new_list.append(
                        mybir.InstEventSemaphore(
                            name=nc.get_next_instruction_name(),
                            ins=[],
                            outs=[],
                            engine=inst.engine,
                            sync_info=mybir.SyncInfo(on_wait=[cond], on_update=[]),
                        )
                    )
                si.on_wait = keep
            new_list.append(inst)
        insts[:] = new_list


_NC_CACHE = None


def _get_nc():
    global _NC_CACHE
    if _NC_CACHE is None:
        _NC_CACHE = _build_program()
    return _NC_CACHE


def _make_in_maps(q, k, v, mask):
    q = np.ascontiguousarray(np.asarray(q, dtype=np.float32))
    k = np.ascontiguousarray(np.asarray(k, dtype=np.float32))
    v = np.ascontiguousarray(np.asarray(v, dtype=np.float32))
    mask = np.asarray(mask)
    negi_np = (np.eye(128, dtype=np.float32) * MASK_NEG).astype(FP8)
    ones_col = np.ones((HPC, S, 1), dtype=np.float32)
    in_maps = []
    for c in range(NCORES):
        sl = slice(c * HPC, (c + 1) * HPC)
        qT = q[sl].transpose(0, 2, 1)  # [HPC, 64, S]
        kT = k[sl].transpose(0, 2, 1)
        qkt_np = np.ascontiguousarray(np.concatenate([qT, kT], axis=2)).astype(ml_dtypes.bfloat16)
        vaug_np = np.ascontiguousarray(np.concatenate([v[sl], ones_col], axis=2)).astype(ml_dtypes.bfloat16)
        mt8_np = np.ascontiguousarray(
            mask[sl].transpose(0, 2, 1).astype(np.float32)
        ).astype(FP8)
        in_maps.append(
            {
                "qkt": qkt_np,
                "vaug": vaug_np,
                "mt8": mt8_np,
                "negi": negi_np,
            }
        )
    return in_maps


def _gather(results):
    outs = []
    for c in range(NCORES):
        outT = np.asarray(results[c]["outt"], dtype=np.float32)  # [HPC, 64, S]
        outs.append(outT.transpose(0, 2, 1))  # [HPC, S, 64]
    return np.ascontiguousarray(np.concatenate(outs, axis=0))


def _install_profile_shim():
    """The agent image's antenv lacks axon_hooks; recreate it from the boot
    module's ctypes implementation so trace=True can capture NTFF profiles."""
    import types

    if "antenv.axon_hooks" in sys.modules:
        return
    try:
        from trn_agent_boot.trn_boot import _ntff_profile_via_ctypes

        hook = _ntff_profile_via_ctypes("/opt/axon/libaxon_pjrt.so")
        mod = types.ModuleType("antenv.axon_hooks")
        mod.get_axon_ntff_profile_hook = lambda: hook
        mod.set_axon_ntff_profile_hook = lambda h: None
        sys.modules["antenv.axon_hooks"] = mod
        # don't try to copy artifacts to a remote bucket from the sandbox
        import concourse.bass_utils as _bu

        _bu.upload_artifacts = lambda tmpdir: tmpdir
    except Exception as e:  # profiling is best-effort
        print(f"profile shim unavailable: {e}", file=sys.stderr)


def run(q, k, v, mask, trace=False, **kw):
    nc = _get_nc()
    if trace:
        _install_profile_shim()
    in_maps = _make_in_maps(q, k, v, mask)
    res = run_bass_kernel_spmd(nc, in_maps, list(range(NCORES)), trace=trace, **kw)
    return _gather(res.results), res


def kernel(q, k, v, mask):
    out, _ = run(q, k, v, mask)
    return out


# revision 11
# speedup vs baseline: 2.7513x; 1.3390x over previous
"""Masked attention kernel for Trainium2, 8 NeuronCores.

Problem: q,k,v [32,1024,64] f32, mask [32,1024,1024] bool (True -> -inf),
out = softmax(q@k^T * D^-0.5 masked) @ v.

Sharding: batch*head dim (32) split across 8 cores, 4 heads/core.

Per-core device algorithm (T-layout):
  scoresT[t,s] = sum_d k[t,d] q[s,d]  computed via PE with
      lhsT = kT [64,128] chunk, rhs = qT [64,1024]  (host provides qT/kT)
  mask added in PSUM via PE:  scoresT += (-240*I128) @ mT  (mask fp8, host-transposed)
      exp(0.125*(s-240)) <= e^-24  -> masked lanes vanish.
  pT = exp(0.125 * scoresT)  on ACT (no row-max needed: |s|*0.125 <= ~6)
  outT_aug[d,s] = sum_t v_aug[t,d] pT[t,s]  with v_aug = [v | ones] so row 64
      carries the softmax denominators.
  out = outT / sums  (reciprocal computed in a transposed [128,8] layout to
      dodge the 8-cycle/elem reciprocal cost, broadcast via tiny DMA bounce)
Host does all pure-layout work: q/k/mask transposes, fp8 cast, final
outT -> out transpose.
"""

import os
import sys

import numpy as np

for _p in ("/opt/trn_rl_repo", "/opt/pypackages"):
    if _p not in sys.path and os.path.isdir(_p):
        sys.path.append(_p)

import ml_dtypes  # noqa: E402

import concourse.bass as bass  # noqa: E402
import concourse.tile as tile  # noqa: E402
from concourse import mybir  # noqa: E402
from concourse.bass_utils import run_bass_kernel_spmd  # noqa: E402

BH, S, D = 32, 1024, 64
NCORES = 8
HPC = BH // NCORES  # heads per core
NT = S // 128  # 8 tiles of 128 along s/t
FP8 = ml_dtypes.float8_e4m3fn
F32 = mybir.dt.float32
BF16 = mybir.dt.bfloat16
DT8 = mybir.dt.float8e4
MASK_NEG = -240.0  # exp(0.125*(-240)) ~ 9e-14; representable in fp8e4m3


def _build_program():
    nc = bass.Bass(
        "TRN2",
        target_bir_lowering=False,
        debug=False,
        num_devices=NCORES,
    )
    qkt = nc.dram_tensor("qkt", [HPC, 64, 2 * S], BF16, kind="ExternalInput").ap()
    vaug = nc.dram_tensor("vaug", [HPC, S, 65], BF16, kind="ExternalInput").ap()
    mt8 = nc.dram_tensor("mt8", [HPC, S, S], DT8, kind="ExternalInput").ap()
    negi = nc.dram_tensor("negi", [128, 128], DT8, kind="ExternalInput").ap()
    ident = nc.dram_tensor("ident", [64, 64], F32, kind="ExternalInput").ap()
    outp = nc.dram_tensor("outp", [HPC, S, D], F32, kind="ExternalOutput").ap()

    with tile.TileContext(nc) as tc:
        with (
            tc.tile_pool(name="const", bufs=1) as const_pool,
            tc.tile_pool(name="qk", bufs=HPC) as qk_pool,
            tc.tile_pool(name="v", bufs=HPC) as v_pool,
            tc.tile_pool(name="m", bufs=HPC) as m_pool,
            tc.tile_pool(name="p", bufs=2) as p_pool,
            tc.tile_pool(name="ot", bufs=2) as ot_pool,
            tc.tile_pool(name="fin", bufs=2) as fin_pool,
            tc.tile_pool(name="spsum", bufs=2, space="PSUM") as s_pool,
            tc.tile_pool(name="opsum", bufs=1, space="PSUM") as o_pool,
            tc.tile_pool(name="fpsum", bufs=1, space="PSUM") as f_pool,
            tc.tile_pool(name="stpsum", bufs=1, space="PSUM") as st_pool,
            tc.tile_pool(name="dram", bufs=2, space="DRAM") as dram_pool,
        ):
            negi_sb = const_pool.tile([128, 128], DT8)
            nc.sync.dma_start(negi_sb[:], negi[:])
            ident_sb = const_pool.tile([64, 64], F32)
            nc.sync.dma_start(ident_sb[:], ident[:])
            ones65_sb = const_pool.tile([65, 1], F32)
            nc.gpsimd.memset(ones65_sb[:], 1.0)

            qk_tiles, v_tiles, m_tiles = [], [], []
            for h in range(HPC):
                qk_sb = qk_pool.tile([64, 2 * S], BF16)
                nc.sync.dma_start(qk_sb[:], qkt[h])
                m_sb = m_pool.tile([128, NT * S], DT8)
                nc.sync.dma_start(
                    m_sb[:].rearrange("p (T s) -> p T s", T=NT),
                    mt8[h].rearrange("(T p) s -> p T s", p=128),
                )
                v_sb = v_pool.tile([128, NT * 65], BF16)
                nc.sync.dma_start(
                    v_sb[:].rearrange("p (T d) -> p T d", T=NT),
                    vaug[h].rearrange("(T p) d -> p T d", p=128),
                )
                qk_tiles.append(qk_sb)
                v_tiles.append(v_sb)
                m_tiles.append(m_sb)

            for h in range(HPC):
                qk_sb, v_sb, m_sb = qk_tiles[h], v_tiles[h], m_tiles[h]
                p_sb = p_pool.tile([128, NT * S], BF16)
                for t in range(NT):
                    s_ps = s_pool.tile([128, S], F32)
                    kslc = slice(S + t * 128, S + (t + 1) * 128)
                    # QK^T (transposed scores): both 512-wide halves with one
                    # weight load, then the mask accumulation with one more.
                    for n in range(2):
                        sl = slice(n * 512, (n + 1) * 512)
                        nc.tensor.matmul(
                            out=s_ps[:, sl],
                            lhsT=qk_sb[:, kslc],
                            rhs=qk_sb[:, sl],
                            start=True,
                            stop=False,
                        )
                    for n in range(2):
                        sl = slice(n * 512, (n + 1) * 512)
                        nc.tensor.matmul(
                            out=s_ps[:, sl],
                            lhsT=negi_sb[:],
                            rhs=m_sb[:, t * S + n * 512 : t * S + (n + 1) * 512],
                            start=False,
                            stop=True,
                        )
                    nc.scalar.activation(
                        out=p_sb[:, t * S : (t + 1) * S],
                        in_=s_ps[:],
                        func=mybir.ActivationFunctionType.Exp,
                        scale=0.125,
                    )

                # out^T augmented with the denominator row (65 = 64 dims + sum)
                o_ps = o_pool.tile([65, S], F32)
                for t in range(NT):
                    for n in range(2):
                        sl = slice(n * 512, (n + 1) * 512)
                        nc.tensor.matmul(
                            out=o_ps[:, sl],
                            lhsT=v_sb[:, t * 65 : (t + 1) * 65],
                            rhs=p_sb[:, t * S + n * 512 : t * S + (n + 1) * 512],
                            start=(t == 0),
                            stop=(t == NT - 1),
                        )
                ot_sb = ot_pool.tile([65, S], F32)
                nc.vector.tensor_copy(ot_sb[:], o_ps[:])

                # transpose sums [1,1024] -> [128,8] via K=1 matmuls (so the
                # 8-cyc/elem reciprocal runs 128 lanes wide), and transpose
                # outT back to [s,d] via PE so the division becomes a cheap
                # per-partition tensor_scalar. Everything stays on-chip.
                st_ps = st_pool.tile([128, NT], F32)
                for j in range(NT):
                    nc.tensor.matmul(
                        out=st_ps[:, j : j + 1],
                        lhsT=ot_sb[64:65, j * 128 : (j + 1) * 128],
                        rhs=ones65_sb[64:65, :],
                        start=True,
                        stop=True,
                    )
                r_sb = fin_pool.tile([128, NT], F32, tag="rsb")
                nc.vector.reciprocal(r_sb[:], st_ps[:])

                f_ps = f_pool.tile([128, NT * D], F32)
                for j in range(NT):
                    nc.tensor.transpose(
                        out=f_ps[:, j * D : (j + 1) * D],
                        in_=ot_sb[0:64, j * 128 : (j + 1) * 128],
                        identity=ident_sb[:],
                    )
                out_sb = fin_pool.tile([128, NT * D], F32, tag="osb")
                for j in range(NT):
                    nc.vector.tensor_scalar_mul(
                        out_sb[:, j * D : (j + 1) * D],
                        in0=f_ps[:, j * D : (j + 1) * D],
                        scalar1=r_sb[:, j : j + 1],
                    )
                nc.sync.dma_start(
                    outp[h].rearrange("(T p) d -> p T d", p=128),
                    out_sb[:].rearrange("p (T d) -> p T d", T=NT),
                )

    _split_multi_waits(nc)
    return nc


def _split_multi_waits(nc):
    """Walrus's S3_LW codegen can't take >1 sync-wait condition on a Matmult;
    hoist extras into standalone EventSemaphore instructions (same semantics:
    the engine queue stalls on them in program order, like raw-bass wait_ge)."""
    for bb in nc.bb_map.values():
        insts = bb.bb.instructions
        new_list = []
        for inst in insts:
            si = getattr(inst, "sync_info", None)
            if (
                si is not None
                and si.on_wait
                and len(si.on_wait) > 1
            ):
                extra = si.on_wait[:-1]
                keep = si.on_wait[-1:]
                for cond in extra:
                    new_list.append(
                        mybir.InstEventSemaphore(
                            name=nc.get_next_instruction_name(),
                            ins=[],
                            outs=[],
                            engine=inst.engine,
                            sync_info=mybir.SyncInfo(on_wait=[cond], on_update=[]),
                        )
                    )
                si.on_wait = keep
            new_list.append(inst)
        insts[:] = new_list


_NC_CACHE = None


def _get_nc():
    global _NC_CACHE
    if _NC_CACHE is None:
        _NC_CACHE = _build_program()
    return _NC_CACHE


def _make_in_maps(q, k, v, mask):
    q = np.ascontiguousarray(np.asarray(q, dtype=np.float32))
    k = np.ascontiguousarray(np.asarray(k, dtype=np.float32))
    v = np.ascontiguousarray(np.asarray(v, dtype=np.float32))
    mask = np.asarray(mask)
    negi_np = (np.eye(128, dtype=np.float32) * MASK_NEG).astype(FP8)
    ident_np = np.eye(64, dtype=np.float32)
    ones_col = np.ones((HPC, S, 1), dtype=np.float32)
    in_maps = []
    for c in range(NCORES):
        sl = slice(c * HPC, (c + 1) * HPC)
        qT = q[sl].transpose(0, 2, 1)  # [HPC, 64, S]
        kT = k[sl].transpose(0, 2, 1)
        qkt_np = np.ascontiguousarray(np.concatenate([qT, kT], axis=2)).astype(ml_dtypes.bfloat16)
        vaug_np = np.ascontiguousarray(np.concatenate([v[sl], ones_col], axis=2)).astype(ml_dtypes.bfloat16)
        mt8_np = np.ascontiguousarray(
            mask[sl].transpose(0, 2, 1).astype(np.float32)
        ).astype(FP8)
        in_maps.append(
            {
                "qkt": qkt_np,
                "vaug": vaug_np,
                "mt8": mt8_np,
                "negi": negi_np,
                "ident": ident_np,
            }
        )
    return in_maps


def _gather(results):
    outs = [np.asarray(results[c]["outp"], dtype=np.float32) for c in range(NCORES)]
    return np.ascontiguousarray(np.concatenate(outs, axis=0))


def _install_profile_shim():
    """The agent image's antenv lacks axon_hooks; recreate it from the boot
    module's ctypes implementation so trace=True can capture NTFF profiles."""
    import types

    if "antenv.axon_hooks" in sys.modules:
        return
    try:
        from trn_agent_boot.trn_boot import _ntff_profile_via_ctypes

        hook = _ntff_profile_via_ctypes("/opt/axon/libaxon_pjrt.so")
        mod = types.ModuleType("antenv.axon_hooks")
        mod.get_axon_ntff_profile_hook = lambda: hook
        mod.set_axon_ntff_profile_hook = lambda h: None
        sys.modules["antenv.axon_hooks"] = mod
        # don't try to copy artifacts to a remote bucket from the sandbox
        import concourse.bass_utils as _bu

        _bu.upload_artifacts = lambda tmpdir: tmpdir
    except Exception as e:  # profiling is best-effort
        print(f"profile shim unavailable: {e}", file=sys.stderr)


def run(q, k, v, mask, trace=False, **kw):
    nc = _get_nc()
    if trace:
        _install_profile_shim()
    in_maps = _make_in_maps(q, k, v, mask)
    res = run_bass_kernel_spmd(nc, in_maps, list(range(NCORES)), trace=trace, **kw)
    return _gather(res.results), res


def kernel(q, k, v, mask):
    out, _ = run(q, k, v, mask)
    return out
